# revision 2
# baseline (speedup 1.0000x reference)
"""Trainium2 Bass kernel for nn_Capsule (dynamic routing) — 8-core data parallel.

Math (per batch b):
  u = x @ W                     # (1024, 25), fp32-ish (f32r matmul)
  b_l = 0
  for it in 0..3:
    c = softmax_n(b_l)          # over the 5 capsules
    o = squash(sum_i c * u)     # (5, 5)
    if it < 3: b_l = einsum(o, u)

Per-core layouts (64 batches/core, groups of 4 batches):
  FAT  partition p = 32*bl + (5n + d), bl in [0,4)      (25 used + 7 pad per slab)
  THIN partition p = 32*gl + (5*bl + n), gl = group-in-set (20 used + 12 pad)
  One "set" = 4 groups = 16 batches; 4 sets per core.

All matmuls run in float32r (HW-measured ~1.6e-4 max rel err, 1 cyc/row).
The squash never normalizes o explicitly during routing: the 1/sqrt(s) factor
is folded into exp's per-partition scale (softmax(rs*b_raw)).
"""
import sys
sys.path.insert(0, '/opt/trn_rl_repo')
import os
import numpy as np
from contextlib import ExitStack

_VPB = int(os.environ.get("KB_VPB", "3"))   # e/rE/c bufs
_XTB = int(os.environ.get("KB_XTB", "4"))   # xt bufs

B_FULL, I_DIM, K_DIM = 512, 1024, 128
N_CORES = 8
B_CORE = B_FULL // N_CORES          # 64
N_CAP, D_CAP = 5, 5
Q_DIM = N_CAP * D_CAP               # 25
N_GROUPS = B_CORE // 4              # 16
N_SETS = N_GROUPS // 4              # 4
ROUTINGS = 4
T_EPS = 1e-7

_CACHE = {}


def _consts():
    """Build the constant 0/1 matrices (host-side, fp32 bits)."""
    s1m = np.zeros((128, 128), np.float32)
    gm = np.zeros((4, 128, 128), np.float32)
    omask = np.zeros((4, 128, 128), np.float32)
    s2m = np.zeros((128, 128), np.float32)
    idm = np.eye(128, dtype=np.float32)

    for gl in range(4):
        for bl in range(4):
            for n in range(N_CAP):
                thin = 32 * gl + 5 * bl + n
                for npr in range(N_CAP):
                    s1m[32 * gl + 5 * bl + npr, thin] = 1.0
                for d in range(D_CAP):
                    fat = 32 * bl + 5 * n + d
                    gm[gl, thin, fat] = 1.0
                    omask[gl, fat, thin] = 1.0
    # pad cols of s1m: give pad rows a positive sumE (copy of (gl, bl=0) sum)
    for gl in range(4):
        for q in range(20, 32):
            for npr in range(N_CAP):
                s1m[32 * gl + npr, 32 * gl + q] = 1.0
    for bl in range(4):
        for n in range(N_CAP):
            for d in range(D_CAP):
                for dp in range(D_CAP):
                    s2m[32 * bl + 5 * n + dp, 32 * bl + 5 * n + d] = 1.0
    return s1m, gm, omask, s2m, idm


def _patch_act_tables():
    # Keep Exp/Ln/Copy only in natural_log_exp_and_others so bacc's
    # table-load pass picks a single resident set (avoids ~27 table
    # reloads, ~2.7us each, from Exp/Ln defaulting to different sets).
    from concourse import bacc as _bacc
    if getattr(_bacc, "_act_tables_patched", False):
        return
    _orig = _bacc.get_activation_tables

    def patched(arch):
        t = _orig(arch)
        keep = "natural_log_exp_and_others"
        if keep in t:
            import concourse.mybir as mybir
            strip = {f for f in t[keep]
                     if f.name in ("Exp", "Ln", "Copy", "Square")}
            for name, fns in t.items():
                if name != keep:
                    t[name] = fns - strip
        return t

    _bacc.get_activation_tables = patched
    _bacc._act_tables_patched = True


def _build_module(repeat: int = 1):
    import concourse.tile as tile
    from concourse import bacc, mybir
    _patch_act_tables()

    F32 = mybir.dt.float32
    F32R = mybir.dt.float32r
    AL = mybir.AluOpType
    AF = mybir.ActivationFunctionType

    nc = bacc.Bacc("TRN2", target_bir_lowering=False, debug=False,
                   enable_asserts=False, num_devices=N_CORES)

    xt_d = nc.dram_tensor("xt", (B_CORE, 128, I_DIM), F32R, kind="ExternalInput").ap()
    w4_d = nc.dram_tensor("w4", (4, 128, 128), F32R, kind="ExternalInput").ap()
    s1m_d = nc.dram_tensor("s1m", (128, 128), F32R, kind="ExternalInput").ap()
    gm_d = nc.dram_tensor("gm", (4, 128, 128), F32R, kind="ExternalInput").ap()
    omf_d = nc.dram_tensor("omf", (4, 128, 128), F32, kind="ExternalInput").ap()
    s2m_d = nc.dram_tensor("s2m", (128, 128), F32, kind="ExternalInput").ap()
    idm_d = nc.dram_tensor("idm", (128, 128), F32, kind="ExternalInput").ap()
    out_d = nc.dram_tensor("out", (N_GROUPS, 4, Q_DIM), F32, kind="ExternalOutput").ap()

    with tile.TileContext(nc) as tc, ExitStack() as ctx:
        singles = ctx.enter_context(tc.tile_pool(name="singles", bufs=1))
        xp = ctx.enter_context(tc.tile_pool(name="xp", bufs=_XTB))
        vp = ctx.enter_context(tc.tile_pool(name="vp", bufs=3))
        pp = ctx.enter_context(tc.tile_pool(name="pp", bufs=1, space="PSUM"))

        # ---- w4 first (phase 1 needs only these + xt) ----
        w4_t = []
        for j in range(4):
            w = singles.tile([128, 128], F32R, name=f"w4_{j}")
            nc.sync.dma_start(w, w4_d[j])
            w4_t.append(w)

        acc_hs = [singles.tile([128, 8], F32, tag=f"acch{s}", name=f"acch_{s}")
                  for s in range(N_SETS)]               # per-half u col-sums
        acc0s = [singles.tile([128, 4], F32, tag=f"acc0{s}", name=f"acc0_{s}")
                 for s in range(N_SETS)]                # iter-0 o_raw (= sum_i u)
        o_all = singles.tile([128, N_GROUPS], F32)      # final-iter o_raw
        uT_tiles = [singles.tile([128, I_DIM], F32R, tag=f"uT{g}", name=f"uT_{g}")
                    for g in range(N_GROUPS)]
        epsb = singles.tile([128, 1], F32)              # squash eps bias
        nc.vector.memset(epsb[:], T_EPS)

        def u_slice(g):
            return uT_tiles[g]

        def phase1(s):
            for gl in range(4):
                g = 4 * s + gl
                xt_t = xp.tile([128, 4 * I_DIM], F32R, tag="xt", name=f"xt_{g}")
                xt_v = xt_t.rearrange("k (bl i) -> k bl i", bl=4)
                if g == 0:
                    for bl in range(4):
                        nc.sync.dma_start(
                            xt_v[:, bl:bl + 1],
                            xt_d[bl:bl + 1].rearrange("bl k i -> k bl i"))
                else:
                    nc.sync.dma_start(
                        xt_v, xt_d[4 * g:4 * g + 4].rearrange("bl k i -> k bl i"))
                for h in range(2):
                    pu = pp.tile([128, 512], F32, tag="pu", bufs=1,
                                 name=f"pu_{g}_{h}")
                    for bl in range(4):
                        nc.tensor.matmul(
                            pu[:], w4_t[bl][:],
                            xt_t[:, bl * I_DIM + 512 * h:bl * I_DIM + 512 * (h + 1)],
                            start=(bl == 0), stop=(bl == 3))
                    nc.scalar.activation(
                        u_slice(g)[:, 512 * h:512 * (h + 1)], pu[:], AF.Copy,
                        accum_out=acc_hs[s][:, 4 * h + gl:4 * h + gl + 1])
            # iter-0 o_raw for this set = h0 + h1 partial sums
            nc.vector.tensor_tensor(
                acc0s[s][:], acc_hs[s][:, 0:4], acc_hs[s][:, 4:8], op=AL.add)

        def load_consts():
            gm_t, omf_t = [], []
            for j in range(4):
                g = singles.tile([128, 128], F32R, name=f"gm_{j}")
                nc.sync.dma_start(g, gm_d[j])
                gm_t.append(g)
                of = singles.tile([128, 128], F32, name=f"omf_{j}")
                nc.sync.dma_start(of, omf_d[j])
                omf_t.append(of)
            s1m_t = singles.tile([128, 128], F32R)
            nc.sync.dma_start(s1m_t, s1m_d)
            s2m_t = singles.tile([128, 128], F32)
            nc.sync.dma_start(s2m_t, s2m_d)
            idm_t = singles.tile([128, 128], F32)
            nc.sync.dma_start(idm_t, idm_d)
            return gm_t, omf_t, s1m_t, s2m_t, idm_t

        consts = {}

        def squash_rs(o_view, o2_tag):
            """o_view: [128,4] fp32 (FAT rows). Returns rs [128,1] at THIN rows."""
            omf_t = consts["omf"]
            o2 = vp.tile([128, 4], F32, tag="o2", name=f"o2_{o2_tag}")
            nc.vector.tensor_tensor(o2[:], o_view, o_view, op=AL.mult)
            sth = pp.tile([128, 512], F32, tag="ps", bufs=3, name=f"sth_{o2_tag}")
            for gl in range(4):
                nc.tensor.matmul(sth[:, 0:1], omf_t[gl][:], o2[:, gl:gl + 1],
                                 start=(gl == 0), stop=(gl == 3))
            lns = vp.tile([128, 1], F32, tag="lns", name=f"lns_{o2_tag}")
            nc.scalar.activation(lns[:], sth[:, 0:1], AF.Ln, bias=epsb[:])
            rs = vp.tile([128, 1], F32, tag="rs", name=f"rs_{o2_tag}")
            nc.scalar.activation(rs[:], lns[:], AF.Exp, scale=-0.5)
            return rs

        def b_update(o_view, s, tag):
            """o_view: [128,4] fp32 FAT. Returns the two b_thin psum halves."""
            omf_t = consts["omf"]
            bths = [pp.tile([128, 512], F32, tag="ps", bufs=3,
                            name=f"bth_{tag}_{h}") for h in range(2)]
            for gl in range(4):
                og = vp.tile([128, 128], F32R, tag="og", bufs=4,
                             name=f"og_{tag}_{gl}")
                nc.vector.tensor_scalar_mul(og[:], omf_t[gl][:], o_view[:, gl:gl + 1])
                for h in range(2):
                    nc.tensor.matmul(bths[h][:], og[:],
                                     u_slice(4 * s + gl)[:, 512 * h:512 * (h + 1)],
                                     start=(gl == 0), stop=(gl == 3))
            return bths

        def routing(s):
            gm_t, s1m_t = consts["gm"], consts["s1m"]
            rs = squash_rs(acc0s[s][:], f"s{s}i0")
            bths = b_update(acc0s[s][:], s, f"s{s}i0")

            for it in range(1, ROUTINGS):
                e_s = vp.tile([128, I_DIM], F32R, tag="e", bufs=_VPB, name=f"e_s{s}i{it}")
                for h in range(2):
                    nc.scalar.activation(e_s[:, 512 * h:512 * (h + 1)],
                                         bths[h][:], AF.Exp, scale=rs[:])
                rE = vp.tile([128, I_DIM], F32, tag="rE", bufs=_VPB, name=f"rE_s{s}i{it}")
                for h in range(2):
                    se = pp.tile([128, 512], F32, tag="ps", bufs=3,
                                 name=f"se_s{s}i{it}_{h}")
                    nc.tensor.matmul(se[:], s1m_t[:], e_s[:, 512 * h:512 * (h + 1)],
                                     start=True, stop=True)
                    nc.vector.reciprocal_approx_fast(
                        out=rE[:, 512 * h:512 * (h + 1)], in_=se[:])
                c_s = vp.tile([128, I_DIM], F32R, tag="c", bufs=_VPB, name=f"c_s{s}i{it}")
                for h in range(2):
                    nc.vector.tensor_tensor(c_s[:, 512 * h:512 * (h + 1)],
                                            e_s.bitcast(F32)[:, 512 * h:512 * (h + 1)],
                                            rE[:, 512 * h:512 * (h + 1)], op=AL.mult)

                o_dst = o_all[:, 4 * s:4 * s + 4] if it == ROUTINGS - 1 else \
                    vp.tile([128, 4], F32, tag="oraw", name=f"oraw_s{s}i{it}")[:]
                dummy = vp.tile([128, 1], F32, tag="dummy",
                                name=f"dum_s{s}i{it}")
                for gl in range(4):
                    cf = pp.tile([128, I_DIM], F32, tag="cfat", bufs=2,
                                 name=f"cf_s{s}i{it}_{gl}")
                    for h in range(2):
                        nc.tensor.matmul(cf[:, 512 * h:512 * (h + 1)], gm_t[gl][:],
                                         c_s[:, 512 * h:512 * (h + 1)],
                                         start=True, stop=True)
                    nc.vector.scalar_tensor_tensor(
                        out=dummy.broadcast_to([128, I_DIM]),
                        in0=cf[:], scalar=1.0,
                        in1=u_slice(4 * s + gl).bitcast(F32)[:],
                        op0=AL.mult, op1=AL.mult,
                        accum_out=o_dst[:, gl:gl + 1])

                if it < ROUTINGS - 1:
                    rs = squash_rs(o_dst, f"s{s}i{it}")
                    bths = b_update(o_dst, s, f"s{s}i{it}")

        # ================= main loop =================
        if repeat > 1:
            consts["gm"], consts["omf"], consts["s1m"], consts["s2m"], \
                consts["idm"] = load_consts()
            rep_cm = tc.For_i(0, repeat, 1)
            rep_cm.__enter__()
            for s in range(N_SETS):
                phase1(s)
                routing(s)
        else:
            rep_cm = None
            phase1(0)
            consts["gm"], consts["omf"], consts["s1m"], consts["s2m"], \
                consts["idm"] = load_consts()
            routing(0)
            for s in range(1, N_SETS):
                phase1(s)
                routing(s)

        # ================= final normalize + output =================
        s2m_t, idm_t = consts["s2m"], consts["idm"]
        o2f = vp.tile([128, N_GROUPS], F32, tag="o2f")
        nc.vector.tensor_tensor(o2f[:], o_all[:], o_all[:], op=AL.mult)
        sf = pp.tile([128, 512], F32, tag="ps", bufs=3)
        nc.tensor.matmul(sf[:, 0:N_GROUPS], s2m_t[:], o2f[:], start=True, stop=True)
        lnf = vp.tile([128, N_GROUPS], F32, tag="lnf")
        nc.scalar.activation(lnf[:], sf[:, 0:N_GROUPS], AF.Ln, bias=epsb[:])
        rsf = vp.tile([128, N_GROUPS], F32, tag="rsf")
        nc.scalar.activation(rsf[:], lnf[:], AF.Exp, scale=-0.5)
        of = vp.tile([128, N_GROUPS], F32, tag="of")
        nc.vector.tensor_tensor(of[:], o_all[:], rsf[:], op=AL.mult)
        pt = pp.tile([N_GROUPS, 512], F32, tag="ps", bufs=3)
        nc.tensor.transpose(pt[:, 0:128], of[:], idm_t[:])
        ot = vp.tile([N_GROUPS, 128], F32, tag="ot")
        nc.scalar.copy(ot[:], pt[:, 0:128])
        nc.sync.dma_start(
            out_d, ot.rearrange("g (bl q) -> g bl q", bl=4)[:, :, 0:Q_DIM])
        if rep_cm is not None:
            rep_cm.__exit__(None, None, None)

    nc.compile()
    return nc


def _get_module(repeat: int = 1):
    key = f"nc{repeat}"
    if key not in _CACHE:
        _CACHE[key] = _build_module(repeat)
    return _CACHE[key]


def _prepare_in_maps(x: np.ndarray, W: np.ndarray) -> list:
    x = np.ascontiguousarray(np.asarray(x, dtype=np.float32))
    W = np.asarray(W, dtype=np.float32)

    s1m, gm, omask, s2m, idm = _consts()
    w4 = np.zeros((4, K_DIM, 128), np.float32)
    for bl in range(4):
        w4[bl, :, 32 * bl:32 * bl + Q_DIM] = W[0]

    xs = x.reshape(N_CORES, B_CORE, I_DIM, K_DIM)
    common = {"w4": w4, "s1m": s1m, "gm": gm, "omf": omask,
              "s2m": s2m, "idm": idm}
    in_maps = []
    for c in range(N_CORES):
        xt = np.ascontiguousarray(xs[c].transpose(0, 2, 1))  # (64, 128, 1024)
        in_maps.append({"xt": xt, **common})
    return in_maps


def kernel(x: np.ndarray, W: np.ndarray) -> np.ndarray:
    from concourse import bass_utils

    in_maps = _prepare_in_maps(x, W)
    nc = _get_module()

    res = bass_utils.run_bass_kernel_spmd(nc, in_maps, core_ids=list(range(N_CORES)))
    outs = []
    for c in range(N_CORES):
        o = res.results[c]["out"]              # (16, 4, 25)
        outs.append(o.reshape(B_CORE, N_CAP, D_CAP))
    return np.concatenate(outs, axis=0)



# revision 16
# speedup vs baseline: 1.0313x; 1.0313x over previous
"""Trainium2 Bass kernel for nn_Capsule (dynamic routing) — 8-core data parallel.

Math (per batch b):
  u = x @ W                     # (1024, 25), fp32-ish (f32r matmul)
  b_l = 0
  for it in 0..3:
    c = softmax_n(b_l)          # over the 5 capsules
    o = squash(sum_i c * u)     # (5, 5)
    if it < 3: b_l = einsum(o, u)

Per-core layouts (64 batches/core, groups of 4 batches):
  FAT  partition p = 32*bl + (5n + d), bl in [0,4)      (25 used + 7 pad per slab)
  THIN partition p = 32*gl + (5*bl + n), gl = group-in-set (20 used + 12 pad)
  One "set" = 4 groups = 16 batches; 4 sets per core.

All matmuls run in float32r (HW-measured ~1.6e-4 max rel err, 1 cyc/row).
The squash never normalizes o explicitly during routing: the 1/sqrt(s) factor
is folded into exp's per-partition scale (softmax(rs*b_raw)).
"""
import sys
sys.path.insert(0, '/opt/trn_rl_repo')
import os
import numpy as np
from contextlib import ExitStack

_VPB = int(os.environ.get("KB_VPB", "3"))   # e/rE/c bufs
_XTB = int(os.environ.get("KB_XTB", "4"))   # xt bufs

B_FULL, I_DIM, K_DIM = 512, 1024, 128
N_CORES = 8
B_CORE = B_FULL // N_CORES          # 64
N_CAP, D_CAP = 5, 5
Q_DIM = N_CAP * D_CAP               # 25
N_GROUPS = B_CORE // 4              # 16
N_SETS = N_GROUPS // 4              # 4
ROUTINGS = 4
T_EPS = 1e-7

_CACHE = {}


def _consts():
    """Build the constant 0/1 matrices (host-side, fp32 bits)."""
    s1m = np.zeros((128, 128), np.float32)
    gm = np.zeros((4, 128, 128), np.float32)
    omask = np.zeros((4, 128, 128), np.float32)
    s2m = np.zeros((128, 128), np.float32)
    idm = np.eye(128, dtype=np.float32)

    for gl in range(4):
        for bl in range(4):
            for n in range(N_CAP):
                thin = 32 * gl + 5 * bl + n
                for npr in range(N_CAP):
                    s1m[32 * gl + 5 * bl + npr, thin] = 1.0
                for d in range(D_CAP):
                    fat = 32 * bl + 5 * n + d
                    gm[gl, thin, fat] = 1.0
                    omask[gl, fat, thin] = 1.0
    # pad cols of s1m: give pad rows a positive sumE (copy of (gl, bl=0) sum)
    for gl in range(4):
        for q in range(20, 32):
            for npr in range(N_CAP):
                s1m[32 * gl + npr, 32 * gl + q] = 1.0
    for bl in range(4):
        for n in range(N_CAP):
            for d in range(D_CAP):
                for dp in range(D_CAP):
                    s2m[32 * bl + 5 * n + dp, 32 * bl + 5 * n + d] = 1.0
    return s1m, gm, omask, s2m, idm


def _patch_act_tables():
    # Keep Exp/Ln/Copy only in natural_log_exp_and_others so bacc's
    # table-load pass picks a single resident set (avoids ~27 table
    # reloads, ~2.7us each, from Exp/Ln defaulting to different sets).
    from concourse import bacc as _bacc
    if getattr(_bacc, "_act_tables_patched", False):
        return
    _orig = _bacc.get_activation_tables

    def patched(arch):
        t = _orig(arch)
        keep = "natural_log_exp_and_others"
        if keep in t:
            import concourse.mybir as mybir
            strip = {f for f in t[keep]
                     if f.name in ("Exp", "Ln", "Copy", "Square")}
            for name, fns in t.items():
                if name != keep:
                    t[name] = fns - strip
        return t

    _bacc.get_activation_tables = patched
    _bacc._act_tables_patched = True


def _build_module(repeat: int = 1):
    import concourse.tile as tile
    from concourse import bacc, mybir
    _patch_act_tables()

    F32 = mybir.dt.float32
    F32R = mybir.dt.float32r
    F16 = mybir.dt.float16
    AL = mybir.AluOpType
    AF = mybir.ActivationFunctionType

    nc = bacc.Bacc("TRN2", target_bir_lowering=False, debug=False,
                   enable_asserts=False, num_devices=N_CORES)

    xt_d = nc.dram_tensor("xt", (B_CORE, 128, I_DIM), F16, kind="ExternalInput").ap()
    w4_d = nc.dram_tensor("w4", (4, 128, 128), F16, kind="ExternalInput").ap()
    s1m_d = nc.dram_tensor("s1m", (128, 128), F32R, kind="ExternalInput").ap()
    gm_d = nc.dram_tensor("gm", (4, 128, 128), F32R, kind="ExternalInput").ap()
    omf_d = nc.dram_tensor("omf", (4, 128, 128), F32, kind="ExternalInput").ap()
    s2m_d = nc.dram_tensor("s2m", (128, 128), F32, kind="ExternalInput").ap()
    idm_d = nc.dram_tensor("idm", (128, 128), F32, kind="ExternalInput").ap()
    out_d = nc.dram_tensor("out", (N_GROUPS, 4, Q_DIM), F32, kind="ExternalOutput").ap()

    with tile.TileContext(nc) as tc, ExitStack() as ctx:
        singles = ctx.enter_context(tc.tile_pool(name="singles", bufs=1))
        xp = ctx.enter_context(tc.tile_pool(name="xp", bufs=_XTB))
        vp = ctx.enter_context(tc.tile_pool(name="vp", bufs=3))
        pp = ctx.enter_context(tc.tile_pool(name="pp", bufs=1, space="PSUM"))

        # ---- w4 first (phase 1 needs only these + xt) ----
        w4_t = []
        for j in range(4):
            w = singles.tile([128, 128], F16, name=f"w4_{j}")
            nc.sync.dma_start(w, w4_d[j])
            w4_t.append(w)

        acc_hs = [singles.tile([128, 8], F32, tag=f"acch{s}", name=f"acch_{s}")
                  for s in range(N_SETS)]               # per-half u col-sums
        acc0s = [singles.tile([128, 4], F32, tag=f"acc0{s}", name=f"acc0_{s}")
                 for s in range(N_SETS)]                # iter-0 o_raw (= sum_i u)
        o_all = singles.tile([128, N_GROUPS], F32)      # final-iter o_raw
        uT_tiles = [singles.tile([128, I_DIM], F32R, tag=f"uT{g}", name=f"uT_{g}")
                    for g in range(N_GROUPS)]
        epsb = singles.tile([128, 1], F32)              # squash eps bias
        nc.vector.memset(epsb[:], T_EPS)

        def u_slice(g):
            return uT_tiles[g]

        def phase1(s):
            for gl in range(4):
                g = 4 * s + gl
                xt_t = xp.tile([128, 4 * I_DIM], F16, tag="xt", name=f"xt_{g}")
                xt_v = xt_t.rearrange("k (bl i) -> k bl i", bl=4)
                if g == 0:
                    for bl in range(4):
                        nc.sync.dma_start(
                            xt_v[:, bl:bl + 1],
                            xt_d[bl:bl + 1].rearrange("bl k i -> k bl i"))
                else:
                    nc.sync.dma_start(
                        xt_v, xt_d[4 * g:4 * g + 4].rearrange("bl k i -> k bl i"))
                for h in range(2):
                    pu = pp.tile([128, 512], F32, tag="pu", bufs=1,
                                 name=f"pu_{g}_{h}")
                    for bl in range(4):
                        nc.tensor.matmul(
                            pu[:], w4_t[bl][:],
                            xt_t[:, bl * I_DIM + 512 * h:bl * I_DIM + 512 * (h + 1)],
                            start=(bl == 0), stop=(bl == 3))
                    nc.scalar.activation(
                        u_slice(g)[:, 512 * h:512 * (h + 1)], pu[:], AF.Copy,
                        accum_out=acc_hs[s][:, 4 * h + gl:4 * h + gl + 1])
            # iter-0 o_raw for this set = h0 + h1 partial sums
            nc.vector.tensor_tensor(
                acc0s[s][:], acc_hs[s][:, 0:4], acc_hs[s][:, 4:8], op=AL.add)

        def load_consts():
            gm_t, omf_t = [], []
            for j in range(4):
                g = singles.tile([128, 128], F32R, name=f"gm_{j}")
                nc.sync.dma_start(g, gm_d[j])
                gm_t.append(g)
                of = singles.tile([128, 128], F32, name=f"omf_{j}")
                nc.sync.dma_start(of, omf_d[j])
                omf_t.append(of)
            s1m_t = singles.tile([128, 128], F32R)
            nc.sync.dma_start(s1m_t, s1m_d)
            s2m_t = singles.tile([128, 128], F32)
            nc.sync.dma_start(s2m_t, s2m_d)
            idm_t = singles.tile([128, 128], F32)
            nc.sync.dma_start(idm_t, idm_d)
            return gm_t, omf_t, s1m_t, s2m_t, idm_t

        consts = {}

        def squash_rs(o_view, o2_tag):
            """o_view: [128,4] fp32 (FAT rows). Returns rs [128,1] at THIN rows."""
            omf_t = consts["omf"]
            o2 = vp.tile([128, 4], F32, tag="o2", name=f"o2_{o2_tag}")
            nc.vector.tensor_tensor(o2[:], o_view, o_view, op=AL.mult)
            sth = pp.tile([128, 512], F32, tag="sth", bufs=1, name=f"sth_{o2_tag}")
            for gl in range(4):
                nc.tensor.matmul(sth[:, 0:1], omf_t[gl][:], o2[:, gl:gl + 1],
                                 start=(gl == 0), stop=(gl == 3))
            lns = vp.tile([128, 1], F32, tag="lns", name=f"lns_{o2_tag}")
            nc.scalar.activation(lns[:], sth[:, 0:1], AF.Ln, bias=epsb[:])
            rs = vp.tile([128, 1], F32, tag="rs", name=f"rs_{o2_tag}")
            nc.scalar.activation(rs[:], lns[:], AF.Exp, scale=-0.5)
            return rs

        def b_update(o_view, s, tag):
            """o_view: [128,4] fp32 FAT. Returns the two b_thin psum halves."""
            omf_t = consts["omf"]
            bths = [pp.tile([128, 512], F32, tag="bths", bufs=2,
                            name=f"bth_{tag}_{h}") for h in range(2)]
            ogs = []
            for gl in range(4):
                og = vp.tile([128, 128], F32R, tag="og", bufs=4,
                             name=f"og_{tag}_{gl}")
                nc.scalar.activation(og[:], omf_t[gl][:], AF.Copy,
                                     scale=o_view[:, gl:gl + 1])
                ogs.append(og)
            for h in range(2):
                for gl in range(4):
                    nc.tensor.matmul(bths[h][:], ogs[gl][:],
                                     u_slice(4 * s + gl)[:, 512 * h:512 * (h + 1)],
                                     start=(gl == 0), stop=(gl == 3))
            return bths

        def routing(s):
            gm_t, s1m_t = consts["gm"], consts["s1m"]
            rs = squash_rs(acc0s[s][:], f"s{s}i0")
            bths = b_update(acc0s[s][:], s, f"s{s}i0")

            for it in range(1, ROUTINGS):
                e_s = vp.tile([128, I_DIM], F32R, tag="e", bufs=_VPB, name=f"e_s{s}i{it}")
                for h in range(2):
                    nc.scalar.activation(e_s[:, 512 * h:512 * (h + 1)],
                                         bths[h][:], AF.Exp, scale=rs[:])
                rE = vp.tile([128, I_DIM], F32, tag="rE", bufs=_VPB, name=f"rE_s{s}i{it}")
                se = pp.tile([128, I_DIM], F32, tag="big", bufs=2,
                             name=f"se_s{s}i{it}")
                for h in range(2):
                    nc.tensor.matmul(se[:, 512 * h:512 * (h + 1)], s1m_t[:],
                                     e_s[:, 512 * h:512 * (h + 1)],
                                     start=True, stop=True)
                nc.vector.reciprocal_approx_fast(out=rE[:], in_=se[:])
                c_s = vp.tile([128, I_DIM], F32R, tag="c", bufs=_VPB, name=f"c_s{s}i{it}")
                for h in range(2):
                    nc.gpsimd.tensor_tensor(
                        c_s[:, 512 * h:512 * (h + 1)],
                        e_s.bitcast(F32)[:, 512 * h:512 * (h + 1)],
                        rE[:, 512 * h:512 * (h + 1)], op=AL.mult)

                o_dst = o_all[:, 4 * s:4 * s + 4] if it == ROUTINGS - 1 else \
                    vp.tile([128, 4], F32, tag="oraw", name=f"oraw_s{s}i{it}")[:]
                dummy = vp.tile([128, 1], F32, tag="dummy",
                                name=f"dum_s{s}i{it}")
                for gl in range(4):
                    cf = pp.tile([128, I_DIM], F32, tag="big", bufs=2,
                                 name=f"cf_s{s}i{it}_{gl}")
                    for h in range(2):
                        nc.tensor.matmul(cf[:, 512 * h:512 * (h + 1)], gm_t[gl][:],
                                         c_s[:, 512 * h:512 * (h + 1)],
                                         start=True, stop=True)
                    nc.vector.scalar_tensor_tensor(
                        out=dummy.broadcast_to([128, I_DIM]),
                        in0=cf[:], scalar=1.0,
                        in1=u_slice(4 * s + gl).bitcast(F32)[:],
                        op0=AL.mult, op1=AL.mult,
                        accum_out=o_dst[:, gl:gl + 1])

                if it < ROUTINGS - 1:
                    rs = squash_rs(o_dst, f"s{s}i{it}")
                    bths = b_update(o_dst, s, f"s{s}i{it}")

        # ================= main loop =================
        if repeat > 1:
            consts["gm"], consts["omf"], consts["s1m"], consts["s2m"], \
                consts["idm"] = load_consts()
            rep_cm = tc.For_i(0, repeat, 1)
            rep_cm.__enter__()
            for s in range(N_SETS):
                phase1(s)
                routing(s)
        else:
            rep_cm = None
            phase1(0)
            consts["gm"], consts["omf"], consts["s1m"], consts["s2m"], \
                consts["idm"] = load_consts()
            routing(0)
            for s in range(1, N_SETS):
                phase1(s)
                routing(s)

        # ================= final normalize + output =================
        s2m_t, idm_t = consts["s2m"], consts["idm"]
        o2f = vp.tile([128, N_GROUPS], F32, tag="o2f")
        nc.vector.tensor_tensor(o2f[:], o_all[:], o_all[:], op=AL.mult)
        sf = pp.tile([128, 512], F32, tag="sth", bufs=1)
        nc.tensor.matmul(sf[:, 0:N_GROUPS], s2m_t[:], o2f[:], start=True, stop=True)
        lnf = vp.tile([128, N_GROUPS], F32, tag="lnf")
        nc.scalar.activation(lnf[:], sf[:, 0:N_GROUPS], AF.Ln, bias=epsb[:])
        rsf = vp.tile([128, N_GROUPS], F32, tag="rsf")
        nc.scalar.activation(rsf[:], lnf[:], AF.Exp, scale=-0.5)
        of = vp.tile([128, N_GROUPS], F32, tag="of")
        nc.vector.tensor_tensor(of[:], o_all[:], rsf[:], op=AL.mult)
        pt = pp.tile([N_GROUPS, 512], F32, tag="bths", bufs=2)
        nc.tensor.transpose(pt[:, 0:128], of[:], idm_t[:])
        ot = vp.tile([N_GROUPS, 128], F32, tag="ot")
        nc.scalar.copy(ot[:], pt[:, 0:128])
        nc.sync.dma_start(
            out_d, ot.rearrange("g (bl q) -> g bl q", bl=4)[:, :, 0:Q_DIM])
        if rep_cm is not None:
            rep_cm.__exit__(None, None, None)

    nc.compile()
    return nc


def _get_module(repeat: int = 1):
    key = f"nc{repeat}"
    if key not in _CACHE:
        _CACHE[key] = _build_module(repeat)
    return _CACHE[key]


def _prepare_in_maps(x: np.ndarray, W: np.ndarray) -> list:
    x = np.ascontiguousarray(np.asarray(x, dtype=np.float32))
    W = np.asarray(W, dtype=np.float32)

    s1m, gm, omask, s2m, idm = _consts()
    w4 = np.zeros((4, K_DIM, 128), np.float16)
    for bl in range(4):
        w4[bl, :, 32 * bl:32 * bl + Q_DIM] = W[0].astype(np.float16)

    xs = x.reshape(N_CORES, B_CORE, I_DIM, K_DIM)
    common = {"w4": w4, "s1m": s1m, "gm": gm, "omf": omask,
              "s2m": s2m, "idm": idm}
    in_maps = []
    for c in range(N_CORES):
        xt = np.ascontiguousarray(
            xs[c].transpose(0, 2, 1).astype(np.float16))  # (64, 128, 1024)
        in_maps.append({"xt": xt, **common})
    return in_maps


def kernel(x: np.ndarray, W: np.ndarray) -> np.ndarray:
    from concourse import bass_utils

    in_maps = _prepare_in_maps(x, W)
    nc = _get_module()

    res = bass_utils.run_bass_kernel_spmd(nc, in_maps, core_ids=list(range(N_CORES)))
    outs = []
    for c in range(N_CORES):
        o = res.results[c]["out"]              # (16, 4, 25)
        outs.append(o.reshape(B_CORE, N_CAP, D_CAP))
    return np.concatenate(outs, axis=0)

